# revision 1
# baseline (speedup 1.0000x reference)
"""Causal single-head attention (B=256, T=256, C=384, H=64) on 8 trn2 cores.

Data-parallel over batch: each core computes bpc=32 batches independently.

v2 design (vs baseline): fp16 matmul dtype (1 cycle/row at any output size,
halves DMA bytes), transposed-softmax formulation (weiT = k^T q with s on
partitions -> no PE transposes at all), natural-layout v projection
(x-stationary), row sums via an appended ones-column in the AV matmul,
causal masking as a multiplicative 0/1 triangle on the Pool engine (SBUF
fp16), and group-batched DMAs (G batches per DMA) to amortize per-DMA
overheads (~625ns HWDGE + ~565ns SEQ each).

Per batch:
  qT[h,t], kT[h,t] = Wq/Wk stationary @ xT moving     (2x3 matmuls, [64,2,256] PSUM)
  v[t,h]           = xT-block stationary @ Wv moving  (6 matmuls, [128,2,64] PSUM)
  weiT[s,t]        = kT-block stationary @ qT moving  (2 matmuls, [128,384] PSUM:
                     cols 0:256 = s0 x all t, cols 256:384 = s1 x t1)
  p = exp(weiT) on ACT (PSUM->SBUF fp16, one instruction; logits ~N(0,1) so no
      max-subtraction needed), diagonal blocks masked by 0/1 upper-triangle
      multiply on Pool.
  out[t, 0:64] + rowsum[t] = p-block stationary @ [v|1] moving (3 matmuls)
  out scaled by 1/rowsum on the PSUM->SBUF copy (DVE for t0, ACT for t1).
"""

import contextlib
import os
import sys

import numpy as np

for _p in ("/opt/trn_rl_repo",):
    if _p not in sys.path:
        sys.path.insert(0, _p)

B, T, C, H = 256, 256, 384, 64
N_CORES = 8
BPC = B // N_CORES  # batches per core
P = 128

LAST_RESULT = None  # BassKernelResults of the most recent run (for test.py)


def _build_nc(bpc=BPC, repeats=1, group=8):
    import concourse.bacc as bacc
    import concourse.mybir as mybir
    import concourse.tile as tile
    from concourse.masks import make_upper_triangular

    f32 = mybir.dt.float32
    f16 = mybir.dt.float16

    G = min(group, bpc)
    assert bpc % G == 0
    NG = bpc // G

    nc = bacc.Bacc("TRN2", target_bir_lowering=False, debug=False)

    xh = nc.dram_tensor("xh", [3, P, bpc, T], f16, kind="ExternalInput")
    wqk = nc.dram_tensor("wqk", [P, 3, P], f16, kind="ExternalInput")
    wv = nc.dram_tensor("wv", [P, 3, H], f16, kind="ExternalInput")
    oh = nc.dram_tensor("oh", [P, bpc, 2, H], f16, kind="ExternalOutput")

    Exp = mybir.ActivationFunctionType.Exp
    Copy = mybir.ActivationFunctionType.Copy
    mult = mybir.AluOpType.mult

    xh_r = xh.rearrange("c p b t -> p c b t")

    with tile.TileContext(nc) as tc:
        with (
            tc.tile_pool(name="consts", bufs=1) as consts,
            tc.tile_pool(name="xg", bufs=2) as xg_pool,
            tc.tile_pool(name="og", bufs=2) as og_pool,
            tc.tile_pool(name="sb", bufs=8) as sb,
            tc.tile_pool(name="ps_a", bufs=3, space="PSUM") as ps_a,
            tc.tile_pool(name="ps_wei", bufs=3, space="PSUM") as ps_wei,
            tc.tile_pool(name="ps_o", bufs=2, space="PSUM") as ps_o,
        ):
            wqk_sb = consts.tile([P, 3, P], f16)
            nc.sync.dma_start(wqk_sb, wqk[:])
            wv_sb = consts.tile([P, 3, H], f16)
            nc.sync.dma_start(wv_sb, wv[:])
            tri01 = consts.tile([P, P], f16)
            make_upper_triangular(nc, tri01, val=1.0, diag=True)
            # manually rotated v_aug buffers: the ones column is written
            # once here; in-loop copies only touch cols 0:H, so it stays
            # 1.0 across all batches and repeat-loop iterations
            vas = []
            for i in range(8):
                va_t = consts.tile([P, 2, H + 1], f16, name=f"va{i}")
                nc.gpsimd.memset(va_t[:, :, H:H + 1], 1.0)
                vas.append(va_t)

            rep_ctx = (
                tc.For_i(0, repeats, 1, hint_engines=(mybir.EngineType.PE,
                                                      mybir.EngineType.DVE,
                                                      mybir.EngineType.Activation,
                                                      mybir.EngineType.Pool,
                                                      mybir.EngineType.SP))
                if repeats > 1
                else contextlib.nullcontext()
            )
            with rep_ctx:
              def emit_tail(pv):
                  """Deferred AV (PE) for batch pv, emitted one batch later."""
                  o_t = ps_o.tile([P, 2, H + 1], f32, tag="o", name="o")
                  pv["o_ps"] = o_t
                  p_sb, v_aug = pv["p"], pv["v_aug"]
                  nc.tensor.matmul(
                      o_t[:, 0, :], p_sb[:, 0:P], v_aug[:, 0, :],
                      start=True, stop=True,
                  )
                  nc.tensor.matmul(
                      o_t[:, 1, :], p_sb[:, P:T], v_aug[:, 0, :],
                      start=True, stop=False,
                  )
                  nc.tensor.matmul(
                      o_t[:, 1, :], p_sb[:, T:3 * P], v_aug[:, 1, :],
                      start=False, stop=True,
                  )

              def emit_norm(pv):
                  o_t = pv["o_ps"]
                  rinv = sb.tile([P, 2], f32, tag="rinv", name="rinv")
                  nc.vector.reciprocal(rinv, o_t[:, :, H])
                  nc.vector.tensor_tensor(
                      pv["og"][:, pv["idx"], :, :], o_t[:, :, 0:H],
                      rinv[:, :, None].to_broadcast((P, 2, H)), mult,
                  )
                  if pv["idx"] == bpc - 1:
                      # single output DMA for the whole iteration
                      nc.sync.dma_start(oh[:], pv["og"])

              prev = None
              cur = {}
              for idx in range(bpc):
                g, j = divmod(idx, G)
                if idx == 0:
                    cur["og"] = og_pool.tile(
                        [P, bpc, 2, H], f16, tag="og", name="og")
                if j == 0:
                    cur["xg"] = xg_pool.tile(
                        [P, 3, G, T], f16, tag="xg", name="xg")
                    nc.sync.dma_start(
                        cur["xg"], xh_r[:, :, g * G:(g + 1) * G, :])
                xg, og = cur["xg"], cur["og"]
                if True:
                    # ---- projections. q|k packed in one 128-wide stationary:
                    # qT lands on PSUM partitions 0:64, kT on 64:128. qk and v
                    # share one PSUM bank (qk bytes 0:1024, v 1024:1536);
                    # their accumulation groups open sequentially in PE
                    # program order and PSUM zeroing is lazy per byte. ----
                    bkA = ps_a.tile([P, 384], f32, tag="bkA")
                    qk_ps = bkA[:, 0:T]
                    v_ps = bkA[:, T:T + P].rearrange("p (i h) -> p i h", i=2)
                    for c in range(3):
                        nc.tensor.matmul(
                            qk_ps, wqk_sb[:, c, :], xg[:, c, j, :],
                            start=(c == 0), stop=(c == 2),
                        )
                    for i in range(2):
                        for c in range(3):
                            nc.tensor.matmul(
                                v_ps[:, i, :],
                                xg[:, c, j, i * P:(i + 1) * P], wv_sb[:, c, :],
                                start=(c == 0), stop=(c == 2),
                            )
                    # deferred AV of batch idx-1 fills the PE gap while this
                    # batch's q/k copies land
                    if prev is not None:
                        emit_tail(prev)

                    # qT (PSUM partitions 0:64) and kT (64:128) both land on
                    # SBUF partitions 0:64; the k copy crosses partitions
                    # (validated on HW) so the weiT matmul sees both operands
                    # at base partition 0.
                    qk_sb = sb.tile([H, 2, T], f16, tag="qk_sb")
                    nc.scalar.copy(qk_sb[:, 0, :], qk_ps[0:H, :])
                    nc.vector.tensor_copy(qk_sb[:, 1, :], qk_ps[H:P, :])
                    v_aug = vas[idx % 8]
                    nc.vector.tensor_copy(v_aug[:, :, 0:H], v_ps)

                    # deferred normalization of batch idx-1 (DVE/SP)
                    if prev is not None:
                        emit_norm(prev)

                    # ---- weiT = k^T q, [s, t] with s on partitions ----
                    wei_ps = ps_wei.tile([P, 3 * P], f32, tag="wei")
                    nc.tensor.matmul(
                        wei_ps[:, 0:T], qk_sb[:, 1, 0:P], qk_sb[:, 0, :],
                        start=True, stop=True,
                    )
                    nc.tensor.matmul(
                        wei_ps[:, T:3 * P], qk_sb[:, 1, P:T], qk_sb[:, 0, P:T],
                        start=True, stop=True,
                    )

                    # ---- softmax numerator (no max subtraction) ----
                    p_sb = sb.tile([P, 3 * P], f16, tag="p")
                    nc.scalar.activation(p_sb, wei_ps, Exp)
                    # causal mask: zero strict-lower triangle of the two
                    # diagonal (s,t) blocks (cols 0:128 and 256:384) in one
                    # strided op
                    p_diag = p_sb.rearrange("p (a q) -> p a q", q=P)[:, 0::2, :]
                    nc.gpsimd.tensor_tensor(
                        p_diag, p_diag,
                        tri01[:, None, :].to_broadcast((P, 2, P)), mult,
                    )

                    prev = {
                        "p": p_sb, "v_aug": v_aug, "og": og, "idx": idx,
                    }

              # drain the final batch
              emit_tail(prev)
              emit_norm(prev)

    nc.compile()
    return nc


def _prep_inputs(x, Wk, Wq, Wv):
    """Full inputs -> per-core in_maps with the DRAM layouts above."""
    x = np.asarray(x, dtype=np.float32)
    scale = np.float32(H) ** np.float32(-0.5)
    wq = np.asarray(Wq, dtype=np.float32) * scale
    wk = np.asarray(Wk, dtype=np.float32)
    wv = np.asarray(Wv, dtype=np.float32)
    # wqk[p, c, 0:64] = Wq_scaled, wqk[p, c, 64:128] = Wk
    wqk_arr = np.concatenate(
        [wq.reshape(3, P, H), wk.reshape(3, P, H)], axis=2
    ).transpose(1, 0, 2)
    wqk_arr = np.ascontiguousarray(wqk_arr.astype(np.float16))
    wv_arr = np.ascontiguousarray(
        wv.reshape(3, P, H).transpose(1, 0, 2).astype(np.float16)
    )
    in_maps = []
    for cid in range(N_CORES):
        xc = x[cid * BPC:(cid + 1) * BPC]  # [bpc, T, C]
        xh = xc.reshape(BPC, T, 3, P).transpose(2, 3, 0, 1)  # [3, P, bpc, T]
        in_maps.append({
            "xh": np.ascontiguousarray(xh.astype(np.float16)),
            "wqk": wqk_arr,
            "wv": wv_arr,
        })
    return in_maps


def _assemble_output(results):
    """Per-core oh [P, bpc, 2, H] fp16 -> full out [B, T, H] fp32."""
    outs = []
    for r in results:
        oh = np.asarray(r["oh"], dtype=np.float32)  # [P, bpc, 2, H]
        outs.append(oh.transpose(1, 2, 0, 3).reshape(BPC, T, H))
    return np.concatenate(outs, axis=0)


def kernel(x, Wk, Wq, Wv):
    global LAST_RESULT
    from concourse.bass_utils import run_bass_kernel_spmd

    in_maps = _prep_inputs(x, Wk, Wq, Wv)
    nc = _build_nc()
    trace = bool(int(os.environ.get("KERNEL_TRACE", "0")))
    res = run_bass_kernel_spmd(
        nc, in_maps, core_ids=list(range(N_CORES)), trace=trace
    )
    LAST_RESULT = res
    return _assemble_output(res.results)



# revision 11
# speedup vs baseline: 1.4467x; 1.4467x over previous
"""Causal single-head attention (B=256, T=256, C=384, H=64) on 8 trn2 cores.

Data-parallel over batch: each core computes bpc=32 batches independently.

v4 design: pair-granular 4-stage software pipeline that WRAPS AROUND the
repeat-loop boundary (no per-iteration fill/drain). All rotating buffers are
persistent tiles indexed by pair parity, so slot s of every iteration runs:

  S1(s):   qk-proj (3 mm, moving x-pair [128,512] -> qkps[s%2]),
           v-proj (12 mm, x-stationary -> W[s%2] pad cols),
           q-copy (ACT), k-copy (DVE, partition-crossing), v-copy (DVE).
  S2(s-1): wei = k^T q per batch (4 mm -> W[s%2] main cols); pair-wide exp
           on ACT (one [128,2,384] op -> p_sb[(s-1)%3]); causal mask as
           0/1-triangle multiply on Pool (SBUF fp16; Pool has no PSUM port).
  S3(s-3): AV (6 mm, ones-column rowsum -> o[(s-3)%2]); fused normalize
           out = o / rowsum via one DVE tensor_tensor divide; per-pair DMA.

Indices (s-1), (s-3) are mod 16: at s<3 they consume the PREVIOUS
iteration's tiles (phantom garbage on iteration 0, overwritten later); an
epilogue after the loop drains the final iteration's last pairs.

PSUM = exactly 8 banks, all bank-aligned: qkps 2x[128,2,256] (2), W
2x[128,2,512] (4; cols 0:384 wei(s-1), 384:512 v(s) -- same-slot writers),
o 2x[128,2,2,128] (2; cols 0:65 used, padded to a full bank).

Engine budget per pair-slot (cost model, PE at full clock): PE 1443ns,
DVE 1442 (k-copy + v-copy + divide-normalize), ACT 1437 (q-copy +
pair-exp), Pool 1111 (mask).
"""

import contextlib
import os
import sys

import numpy as np

for _p in ("/opt/trn_rl_repo",):
    if _p not in sys.path:
        sys.path.insert(0, _p)

B, T, C, H = 256, 256, 384, 64
N_CORES = 8
BPC = B // N_CORES  # batches per core
NP = BPC // 2       # pairs per core
P = 128

LAST_RESULT = None  # BassKernelResults of the most recent run (for test.py)


def _build_nc(bpc=BPC, repeats=1, group=4, unroll=1, lag2=2, lag3=4,
              nps=4, nv=5, nqs=3):
    import concourse.bacc as bacc
    import concourse.mybir as mybir
    import concourse.tile as tile
    from concourse.masks import make_upper_triangular

    f32 = mybir.dt.float32
    f16 = mybir.dt.float16

    np_ = bpc // 2            # pairs per core
    G = min(group, bpc)       # batches per DMA group
    assert bpc % G == 0 and G % 2 == 0
    PPG = G // 2              # pairs per DMA group
    NG = bpc // G             # DMA groups

    nc = bacc.Bacc("TRN2", target_bir_lowering=False, debug=False)

    xh = nc.dram_tensor("xh", [3, P, bpc, T], f16, kind="ExternalInput")
    wqk = nc.dram_tensor("wqk", [P, 3, P], f16, kind="ExternalInput")
    wv = nc.dram_tensor("wv", [P, 3, H], f16, kind="ExternalInput")
    oh = nc.dram_tensor("oh", [P, bpc, 2, H], f16, kind="ExternalOutput")

    Exp = mybir.ActivationFunctionType.Exp
    mult = mybir.AluOpType.mult
    divide = mybir.AluOpType.divide

    xh_r = xh.rearrange("c p b t -> p c b t")

    NPS = nps  # p_sb rotation depth
    NV = nv    # v_aug rotation depth

    with tile.TileContext(nc) as tc:
        with (
            tc.tile_pool(name="consts", bufs=1) as consts,
            tc.tile_pool(name="psum", bufs=1, space="PSUM") as psum,
        ):
            # ---- persistent PSUM tiles (8 banks exactly, bank-aligned) ----
            qkps = [psum.tile([P, 2, T], f32, name=f"qkps{i}")
                    for i in range(2)]
            Ws = [psum.tile([P, 2, 4 * P], f32, name=f"W{i}")
                  for i in range(2)]
            os_ = [psum.tile([P, 2, 2, P], f32, name=f"o{i}")
                   for i in range(2)]

            # ---- constants ----
            wqk_sb = consts.tile([P, 3, P], f16)
            nc.sync.dma_start(wqk_sb, wqk[:])
            wv_sb = consts.tile([P, 3, H], f16)
            nc.sync.dma_start(wv_sb, wv[:])
            tri01 = consts.tile([P, P], f16)
            make_upper_triangular(nc, tri01, val=1.0, diag=True)

            # ---- persistent SBUF rotations (memset so iteration-0 phantom
            # stages read finite values) ----
            xgs = [consts.tile([P, 3, G, T], f16, name=f"xg{i}")
                   for i in range(2)]
            qk_sbs = []
            for i in range(nqs):
                t_ = consts.tile([H, 2, 2, T], f16, name=f"qksb{i}")
                nc.gpsimd.memset(t_, 0.0)
                qk_sbs.append(t_)
            p_sbs = []
            for i in range(NPS):
                t_ = consts.tile([P, 2, 3 * P], f16, name=f"psb{i}")
                nc.gpsimd.memset(t_, 0.0)
                p_sbs.append(t_)
            vas = []
            for i in range(NV):
                t_ = consts.tile([P, 2, 2, H + 1], f16, name=f"va{i}")
                nc.gpsimd.memset(t_, 0.0)
                nc.gpsimd.memset(t_[:, :, :, H:H + 1], 1.0)
                vas.append(t_)
            ogs = [consts.tile([P, 2, 2, H], f16, name=f"og{i}")
                   for i in range(2)]
            rinvs = [consts.tile([P, 2, 2], f32, name=f"rinv{i}")
                     for i in range(2)]

            def emit_s2_mm(X):
                """wei matmuls for pair X (runs at slot X+2)."""
                qk_sb = qk_sbs[X % nqs]
                Wt = Ws[(X + lag2) % 2]
                for b in range(2):
                    nc.tensor.matmul(
                        Wt[:, b, 0:T], qk_sb[:, b, 1, 0:P], qk_sb[:, b, 0, :],
                        start=True, stop=True,
                    )
                    nc.tensor.matmul(
                        Wt[:, b, T:3 * P], qk_sb[:, b, 1, P:T],
                        qk_sb[:, b, 0, P:T],
                        start=True, stop=True,
                    )

            def emit_s2_post(X):
                """exp + mask for pair X."""
                Wt = Ws[(X + lag2) % 2]
                p_sb = p_sbs[X % NPS]
                # pair-wide exp (one ACT instruction, strided over b)
                nc.scalar.activation(p_sb, Wt[:, :, 0:3 * P], Exp)
                # causal mask: zero strict-lower triangle of the diagonal
                # (s,t) blocks (free cols 0:128 and 256:384 per batch)
                p_diag = p_sb.rearrange(
                    "p b (a q) -> p b a q", q=P)[:, :, 0::2, :]
                nc.gpsimd.tensor_tensor(
                    p_diag, p_diag,
                    tri01[:, None, None, :].to_broadcast((P, 2, 2, P)), mult,
                )

            def emit_s2(X):
                emit_s2_mm(X)
                emit_s2_post(X)

            def emit_s3(X):
                """AV + fused normalize + output DMA for pair X (slot X+4)."""
                o_t = os_[X % 2]
                p_sb, va = p_sbs[X % NPS], vas[X % NV]
                for b in range(2):
                    nc.tensor.matmul(
                        o_t[:, b, 0, 0:H + 1], p_sb[:, b, 0:P], va[:, b, 0, :],
                        start=True, stop=True,
                    )
                    nc.tensor.matmul(
                        o_t[:, b, 1, 0:H + 1], p_sb[:, b, P:T], va[:, b, 0, :],
                        start=True, stop=False,
                    )
                    nc.tensor.matmul(
                        o_t[:, b, 1, 0:H + 1], p_sb[:, b, T:3 * P],
                        va[:, b, 1, :],
                        start=False, stop=True,
                    )
                og = ogs[X % 2]
                # a TT op may read only ONE input from PSUM: reciprocal the
                # rowsums into SBUF first, then multiply
                rinv = rinvs[X % 2]
                nc.vector.reciprocal(rinv, o_t[:, :, :, H])
                nc.vector.tensor_tensor(
                    og, o_t[:, :, :, 0:H],
                    rinv[:, :, :, None].to_broadcast((P, 2, 2, H)),
                    mult,
                )
                nc.sync.dma_start(oh[:, 2 * X:2 * X + 2, :, :], og)

            def emit_body():
                for s in range(np_):
                    g, j = divmod(s, PPG)
                    if j == 0:
                        # prefetch the NEXT group (wraps to the next
                        # iteration's group 0 at the end)
                        gn = (g + 1) % NG
                        nc.sync.dma_start(
                            xgs[gn % 2], xh_r[:, :, gn * G:(gn + 1) * G, :])
                    xg = xgs[g % 2]

                    # ---- S1(s): projections ----
                    qk_ps = qkps[s % 2]
                    Wt = Ws[s % 2]
                    for c in range(3):
                        nc.tensor.matmul(
                            qk_ps, wqk_sb[:, c, :],
                            xg[:, c, 2 * j:2 * j + 2, :],
                            start=(c == 0), stop=(c == 2),
                        )
                    for b in range(2):
                        for i in range(2):
                            for c in range(3):
                                nc.tensor.matmul(
                                    Wt[:, b, 3 * P + i * H:3 * P + (i + 1) * H],
                                    xg[:, c, 2 * j + b, i * P:(i + 1) * P],
                                    wv_sb[:, c, :],
                                    start=(c == 0), stop=(c == 2),
                                )

                    # emission order: wei matmuls BEFORE the copies (the
                    # W banks are shared between wei(s-lag2) and v(s); PSUM
                    # dep tracking is bank-granular, so v-copy(s) emitted
                    # before wei would give wei a false WAR on it), but
                    # q-copy stays first in the ACT stream (exp after).
                    X2 = (s - lag2) % np_
                    emit_s2_mm(X2)

                    # copies for pair s
                    qk_sb = qk_sbs[s % nqs]
                    nc.scalar.copy(qk_sb[:, :, 0, :], qk_ps[0:H, :, :])
                    nc.vector.tensor_copy(qk_sb[:, :, 1, :], qk_ps[H:P, :, :])
                    va = vas[s % NV]
                    nc.vector.tensor_copy(
                        va[:, :, :, 0:H],
                        Wt[:, :, 3 * P:4 * P].rearrange(
                            "p b (i h) -> p b i h", i=2),
                    )

                    emit_s2_post(X2)
                    emit_s3((s - lag3) % np_)

            # prologue: first input group
            nc.sync.dma_start(xgs[0], xh_r[:, :, 0:G, :])

            if repeats > 1:
                with tc.For_i(0, repeats, 1,
                              hint_engines=(mybir.EngineType.PE,
                                            mybir.EngineType.DVE,
                                            mybir.EngineType.Activation,
                                            mybir.EngineType.Pool,
                                            mybir.EngineType.SP)):
                    emit_body()
            else:
                for _ in range(unroll):
                    emit_body()

            # epilogue: drain the final iteration's tail pairs
            for X in range(np_ - lag2, np_):
                emit_s2(X)
            for X in range(np_ - lag3, np_):
                emit_s3(X)

    nc.compile()
    return nc


def _prep_inputs(x, Wk, Wq, Wv):
    """Full inputs -> per-core in_maps with the DRAM layouts above."""
    x = np.asarray(x, dtype=np.float32)
    scale = np.float32(H) ** np.float32(-0.5)
    wq = np.asarray(Wq, dtype=np.float32) * scale
    wk = np.asarray(Wk, dtype=np.float32)
    wv = np.asarray(Wv, dtype=np.float32)
    # wqk[p, c, 0:64] = Wq_scaled, wqk[p, c, 64:128] = Wk
    wqk_arr = np.concatenate(
        [wq.reshape(3, P, H), wk.reshape(3, P, H)], axis=2
    ).transpose(1, 0, 2)
    wqk_arr = np.ascontiguousarray(wqk_arr.astype(np.float16))
    wv_arr = np.ascontiguousarray(
        wv.reshape(3, P, H).transpose(1, 0, 2).astype(np.float16)
    )
    in_maps = []
    for cid in range(N_CORES):
        xc = x[cid * BPC:(cid + 1) * BPC]  # [bpc, T, C]
        xh = xc.reshape(BPC, T, 3, P).transpose(2, 3, 0, 1)  # [3, P, bpc, T]
        in_maps.append({
            "xh": np.ascontiguousarray(xh.astype(np.float16)),
            "wqk": wqk_arr,
            "wv": wv_arr,
        })
    return in_maps


def _assemble_output(results):
    """Per-core oh [P, bpc, 2, H] fp16 -> full out [B, T, H] fp32."""
    outs = []
    for r in results:
        oh = np.asarray(r["oh"], dtype=np.float32)  # [P, bpc, 2, H]
        outs.append(oh.transpose(1, 2, 0, 3).reshape(BPC, T, H))
    return np.concatenate(outs, axis=0)


def kernel(x, Wk, Wq, Wv):
    global LAST_RESULT
    from concourse.bass_utils import run_bass_kernel_spmd

    in_maps = _prep_inputs(x, Wk, Wq, Wv)
    nc = _build_nc()
    trace = bool(int(os.environ.get("KERNEL_TRACE", "0")))
    res = run_bass_kernel_spmd(
        nc, in_maps, core_ids=list(range(N_CORES)), trace=trace
    )
    LAST_RESULT = res
    return _assemble_output(res.results)
